# revision 14
# baseline (speedup 1.0000x reference)
"""BigResNet Trainium2 kernel.

Computation (see reference): x:[65536,100]; 100 blocks of
(10x Linear(100,100)+ReLU) with a residual add per block; final Linear(100,10).

Strategy:
- Data-parallel over the batch: 8 cores x 8192 rows each.
- Activations live in SBUF transposed: [D=100 (+1 ones row), batch]. The
  contraction dim D sits on SBUF partitions for both matmul operands, so no
  transposes are needed anywhere in the layer chain.
- Bias is folded into the matmul via a constant ones-row at partition 100 and
  an extra weight row (K=101).
- Weights are host-side rearranged to [101, block, layer*100] so each block's
  weights DMA as 101 partitions x 4000B contiguous lines.
- Matmul dtype float32r (fp32 truncated to FP22 inside the PE): full PE rate,
  ~2^-12 relative precision, fp32 accumulate in PSUM.
- ReLU drains PSUM->SBUF split between ScalarE (activation) and VectorE
  (tensor_scalar_max); these two engines are the hard bottleneck (GpSimd and
  DMA have no PSUM port on TRN2).
- Block residual is folded into the matmuls: the first layer of block b+1
  computes W1*(r_b + x_b) as two PSUM-accumulated matmuls W1*r_b + W1*x_b
  (the x buffer's ones-row is 0 so the bias isn't double counted and both
  matmuls share one LDWEIGHTS). The actual x_{b+1} = r_b + x_b is
  materialized in-place on GpSimd off the critical path. This removes all
  residual adds from the ACT/DVE drain path and kills the per-block PE stall
  (and with it the HAM re-throttle cold penalty seen in the old trace).
- The final Linear uses the same trick: out = Wf*r_99 + Wf*x_99 + bf.
"""

import sys

sys.path.insert(0, "/opt/trn_rl_repo")

import numpy as np
from contextlib import ExitStack

import concourse.bass as bass
import concourse.bacc as bacc
import concourse.tile as tile
from concourse import mybir
from concourse.bass_utils import run_bass_kernel_spmd
from concourse import bass_utils as _bu


def _enable_ldw_opt():
    """walrus ships with --enable-ldw-opt=false; our inner loop issues 16+
    matmuls per weight load, so redundant LDWEIGHTS cost ~80ns/matmul.
    Rewrite the flag on the walrus command line."""
    if getattr(_bu, "_ldw_opt_patched", False):
        return
    _orig = _bu.run_command

    def run_command(cmd, *a, **k):
        cmd = ["--enable-ldw-opt=true" if c == "--enable-ldw-opt=false" else c
               for c in cmd]
        return _orig(cmd, *a, **k)

    _bu.run_command = run_command
    _bu._ldw_opt_patched = True


_enable_ldw_opt()

N_BLOCKS = 100
LAYERS_PER_BLOCK = 10
D = 100
D_OUT = 10
BATCH = 65536
N_CORES = 8
B_CORE = BATCH // N_CORES  # 8192 batch columns per core
KAUG = D + 1  # 100 weight rows + 1 bias row

F32 = mybir.dt.float32
F32R = mybir.dt.float32r

# Column-group size for the PSUM->SBUF drain ops (ReLU). PSUM is 8 banks of
# 512 fp32; [100,1024] tiles x 4 bufs fill it exactly and keep 4 chunks in
# flight so the PE never waits on banks.
GROUP = 1024
N_GROUPS = B_CORE // GROUP  # 8
MM_N = 512  # max moving-operand free dim for fp32
MM_PER_GROUP = GROUP // MM_N  # 2


# Per (global-layer, group) ReLU engine assignment: even groups on ScalarE
# (ACT ~1116ns/1024), odd on VectorE (DVE ~1215ns/1024). ACT is slightly
# faster, so it also picks up group 7 on two layers out of ten to land at
# 42/38 chunks per block.
def _use_act(gl: int, group: int) -> bool:
    if group % 2 == 0:
        return True
    return group == 7 and gl % 10 in (3, 7)


def _build(n_blocks: int = N_BLOCKS, b_core: int = B_CORE):
    n_groups = b_core // GROUP
    nc = bacc.Bacc("TRN2", target_bir_lowering=False, debug=False,
                   num_devices=N_CORES)

    xt = nc.dram_tensor("xt", [KAUG, b_core], F32R, kind="ExternalInput").ap()
    zrow = nc.dram_tensor("zrow", [1, b_core], F32R, kind="ExternalInput").ap()
    wa = nc.dram_tensor("wa", [KAUG, n_blocks, LAYERS_PER_BLOCK * D], F32R,
                        kind="ExternalInput").ap()
    wf = nc.dram_tensor("wf", [KAUG, D_OUT], F32R, kind="ExternalInput").ap()
    out = nc.dram_tensor("out", [D_OUT, b_core], F32,
                         kind="ExternalOutput").ap()

    with tile.TileContext(nc) as tc, ExitStack() as ctx:
        acts = ctx.enter_context(tc.tile_pool(name="acts", bufs=1))
        wpool = ctx.enter_context(tc.tile_pool(name="w", bufs=2))
        wfpool = ctx.enter_context(tc.tile_pool(name="wf", bufs=1))
        opool = ctx.enter_context(tc.tile_pool(name="o", bufs=1))
        psum = ctx.enter_context(tc.tile_pool(name="ps", bufs=4, space="PSUM"))

        # Three rotating activation buffers (ones-row = 1) plus the residual
        # carry buffer X (ones-row = 0 so block-boundary double-matmuls don't
        # apply the bias twice).
        tbuf = [acts.tile([KAUG, b_core], F32R, tag=f"act{i}", name=f"act{i}")
                for i in range(3)]
        xres = acts.tile([KAUG, b_core], F32R, tag="xres", name="xres")
        # x lands in tbuf[0]; host ships the ones-row as row 100 of xt.
        nc.gpsimd.dma_start(tbuf[0][:, :], xt[:, :])
        nc.gpsimd.dma_start(tbuf[1][D:KAUG, :], xt[D:KAUG, :])
        nc.gpsimd.dma_start(tbuf[2][D:KAUG, :], xt[D:KAUG, :])
        # X starts as a second copy of the input, with a zero ones-row
        # (f32r memset is not ISA-supported; DMA a host zero row instead).
        nc.gpsimd.dma_start(xres[0:D, :], xt[0:D, :])
        nc.gpsimd.dma_start(xres[D:KAUG, :], zrow[:, :])

        wf_sb = wfpool.tile([KAUG, D_OUT], F32R)
        nc.gpsimd.dma_start(wf_sb[:, :], wf[:, :])

        ri = 0  # index of the buffer holding r_{b-1} (block input residual)
        N_PRE = 4  # next-block layer-1 chunks whose W1*x matmul is hoisted
        wt = wpool.tile([KAUG, LAYERS_PER_BLOCK * D], F32R, tag="wt")
        nc.gpsimd.dma_start(wt[:, :], wa[:, 0, :])
        pre_ps = None
        for bl in range(n_blocks):
            r = tbuf[ri]
            d1 = tbuf[(ri + 1) % 3]
            d2 = tbuf[(ri + 2) % 3]

            cur = r
            for layer in range(LAYERS_PER_BLOCK):
                gl = bl * LAYERS_PER_BLOCK + layer
                w_l = wt[:, layer * D:(layer + 1) * D]
                dst = d1 if layer % 2 == 0 else d2
                double = layer == 0 and bl > 0
                for g in range(n_groups):
                    if double and pre_ps is not None and g < N_PRE:
                        # W1*x was already accumulated during the previous
                        # block's last layer (as a closed matmul group); add
                        # W1*r on top via has_written accumulation.
                        ps = pre_ps[g]
                        for h in range(MM_PER_GROUP):
                            c0 = g * GROUP + h * MM_N
                            nc.tensor.matmul(
                                ps[:, h * MM_N:(h + 1) * MM_N], w_l,
                                cur[:, c0:c0 + MM_N],
                                start=False, stop=True,
                                skip_group_check=True)
                    else:
                        ps = psum.tile([D, GROUP], F32, tag="ps")
                        for h in range(MM_PER_GROUP):
                            c0 = g * GROUP + h * MM_N
                            if double:
                                # z = W1*r + W1*x (= W1*(r+x)); shared
                                # stationary, so one LDWEIGHTS serves both.
                                nc.tensor.matmul(
                                    ps[:, h * MM_N:(h + 1) * MM_N], w_l,
                                    xres[:, c0:c0 + MM_N],
                                    start=True, stop=False)
                                nc.tensor.matmul(
                                    ps[:, h * MM_N:(h + 1) * MM_N], w_l,
                                    cur[:, c0:c0 + MM_N],
                                    start=False, stop=True)
                            else:
                                nc.tensor.matmul(
                                    ps[:, h * MM_N:(h + 1) * MM_N], w_l,
                                    cur[:, c0:c0 + MM_N],
                                    start=True, stop=True)
                    gs = slice(g * GROUP, (g + 1) * GROUP)
                    if _use_act(gl, g):
                        nc.scalar.activation(
                            dst[0:D, gs], ps[:, :],
                            mybir.ActivationFunctionType.Relu)
                    else:
                        nc.vector.tensor_scalar_max(dst[0:D, gs], ps[:, :], 0.0)
                cur = dst
                if layer == 0 and bl > 0:
                    # Materialize x_b = r_{b-1} + x_{b-1} in place, off the
                    # drain path. Needed only by block b+1's first layer.
                    nc.gpsimd.tensor_add(xres[0:D, :], xres[0:D, :], r[0:D, :])
                if layer == LAYERS_PER_BLOCK - 3 and bl < n_blocks - 1:
                    # Prefetch next block's weights early so the hoisted
                    # matmuls below don't wait on the DMA.
                    wt_next = wpool.tile([KAUG, LAYERS_PER_BLOCK * D], F32R,
                                         tag="wt")
                    nc.gpsimd.dma_start(wt_next[:, :], wa[:, bl + 1, :])
                if layer == LAYERS_PER_BLOCK - 1 and bl < n_blocks - 1:
                    # Pre-open the next block's first-layer leading PSUM
                    # chunks with the W1*x half of the boundary double-
                    # matmul. This fills the PE's idle tail here and halves
                    # the PE burst at the block boundary, which otherwise
                    # starves the drain engines.
                    wt = wt_next
                    w_n0 = wt[:, 0:D]
                    pre_ps = [psum.tile([D, GROUP], F32, tag="ps",
                                        name=f"pre_ps{i}")
                              for i in range(N_PRE)]
                    for g in range(N_PRE):
                        for h in range(MM_PER_GROUP):
                            c0 = g * GROUP + h * MM_N
                            nc.tensor.matmul(
                                pre_ps[g][:, h * MM_N:(h + 1) * MM_N], w_n0,
                                xres[:, c0:c0 + MM_N],
                                start=True, stop=True,
                                skip_group_check=True)
            # new r is d2 (layer 10's dst)
            ri = (ri + 2) % 3

        # Final Linear(100 -> 10): out = Wf*r_99 + Wf*x_99 + bf.
        r = tbuf[ri]
        out_sb = opool.tile([D_OUT, b_core], F32)
        for g in range(n_groups):
            ps = psum.tile([D_OUT, GROUP], F32, tag="ps")
            for h in range(MM_PER_GROUP):
                c0 = g * GROUP + h * MM_N
                nc.tensor.matmul(ps[:, h * MM_N:(h + 1) * MM_N], wf_sb[:, :],
                                 r[:, c0:c0 + MM_N], start=True, stop=False)
                nc.tensor.matmul(ps[:, h * MM_N:(h + 1) * MM_N], wf_sb[:, :],
                                 xres[:, c0:c0 + MM_N], start=False, stop=True)
            gs = slice(g * GROUP, (g + 1) * GROUP)
            if g % 2 == 0:
                nc.vector.tensor_copy(out_sb[:, gs], ps[:, :])
            else:
                nc.scalar.copy(out_sb[:, gs], ps[:, :])
        nc.gpsimd.dma_start(out[:, :], out_sb[:, :])

    nc.compile()
    return nc


def _prep_inputs(x, W, b, Wf, bf):
    """Host-side reshape/augment; returns per-core input maps."""
    # wa[i, bl, l*100+o]: i<100 -> W[bl,l,o,i]; i==100 -> b[bl,l,o]
    wa = np.empty((KAUG, N_BLOCKS, LAYERS_PER_BLOCK * D), np.float32)
    wt = np.ascontiguousarray(W.transpose(3, 0, 1, 2))  # [i, bl, l, o]
    wa[:D] = wt.reshape(D, N_BLOCKS, LAYERS_PER_BLOCK * D)
    wa[D] = b.reshape(N_BLOCKS, LAYERS_PER_BLOCK * D)

    wfa = np.empty((KAUG, D_OUT), np.float32)
    wfa[:D] = Wf.T
    wfa[D] = bf

    xt = np.empty((KAUG, BATCH), np.float32)
    xt[:D] = x.T
    xt[D] = 1.0

    in_maps = []
    for c in range(N_CORES):
        sl = slice(c * B_CORE, (c + 1) * B_CORE)
        in_maps.append({
            "xt": np.ascontiguousarray(xt[:, sl]),
            "zrow": np.zeros((1, B_CORE), np.float32),
            "wa": wa,
            "wf": wfa,
        })
    return in_maps


_CACHED_NC = None


def kernel(x, W, b, Wf, bf, _trace=False, _trace_kwargs=None):
    global _CACHED_NC
    x = np.asarray(x, np.float32)
    in_maps = _prep_inputs(np.asarray(x, np.float32), np.asarray(W, np.float32),
                           np.asarray(b, np.float32), np.asarray(Wf, np.float32),
                           np.asarray(bf, np.float32))
    if _CACHED_NC is None:
        _CACHED_NC = _build()
    nc = _CACHED_NC
    kw = dict(_trace_kwargs or {})
    res = run_bass_kernel_spmd(nc, in_maps, core_ids=list(range(N_CORES)),
                               trace=_trace, **kw)
    outs = [res.results[c]["out"] for c in range(N_CORES)]  # [10, 8192] each
    full = np.concatenate(outs, axis=1).T  # [65536, 10]
    if _trace:
        kernel.last_results = res
    return np.ascontiguousarray(full)
